# revision 24
# baseline (speedup 1.0000x reference)
"""AdaptiveTokenMixer Trainium2 kernel (8 NeuronCores, pure data parallel).

Per-core algorithm (one batch element per core), pipelined in 4 chunks:
  1. alpha stage runs in [105, 320] layout (partition q = (third h, block b),
     free = (pos i, tap p)); sliding windows are overlapping-stride AP reads
     of a single packed dt/valid/bw row tensor -- no PE transposes.
     exp(-td-12) temporal-decay weights (the bias keeps unmasked pad taps
     finite and cancels in the normalization); the softmax denominator is
     folded into the blend: alpha ~ (e + cv*bw*s) / sum, where
     bw = (b/(1-b))*softmax(w) is host-precomputed.
  2. Three skewed DMAs per chunk (one per third, queues alternate by chunk)
     scatter alpha (bf16) into a DRAM scratch forming the banded
     W^T[m, k] = alpha[n0+m, k-m] per 120-position block.
  3. One DMA-transpose XBAR per chunk (Sync queue only -- the XBAR
     misbehaves on the Act queue) loads its W[k, m] blocks into SBUF;
     one 128x120 @ 128x256 bf16 matmul per block realizes the K-tap mixing
     exactly (PSUM f32, two blocks share a PSUM bank).
  4. Paired PSUM evictions to bf16 staging (DVE/ACT alternating); chunked
     stores overlap the remaining matmuls.
DMA instruction count and order are deliberate: the Tile scheduler
round-robins 8 HWDGE semaphores over all DMAs, so extra or reordered DMAs
couple unrelated phases through semaphore-reuse serialization.

Self-contained: hardcodes shapes for B=8, N=4096, d=256, K=8.
"""
import numpy as np
import ml_dtypes

import concourse.bass as bass
import concourse.bacc as bacc
import concourse.mybir as mybir
from concourse import tile
from concourse.bass_utils import run_bass_kernel_spmd

B, N, D, K = 8, 4096, 256, 8
BLK = 120                      # output positions per block
NB = (N + BLK - 1) // BLK      # 35 blocks -> covers 4200 positions
NOUT = NB * BLK                # 4200 rows in padded device output
NPAD = 4224                    # padded input length
KW = 128                       # k-window (contraction) per block
WBLK = KW * KW                 # W scratch elements per block
NH = 3                         # thirds per block in alpha layout
IH = BLK // NH                 # 40 positions per third
QP = NH * NB                   # 105 partitions used in alpha stage
FA = IH * K                    # 320 free elements per alpha partition
ROWW = IH + K                  # 48: dt/vf row width per third
CW = 2 * ROWW + K + 1          # 105: packed comb row (dt | vf | bw | ebias)
CHUNKS = [(0, 5), (5, 10), (15, 10), (25, 10)]
XCHUNKS = [(0, 18), (18, 17)]            # x load: one chunk per queue
SCHUNKS = [(0, 12), (12, 12), (24, 11)]  # out-store chunking
EBIAS = -12.0                  # exp bias: cancels in softmax, avoids overflow

_CACHE = {}


def _build():
    nc = bacc.Bacc("TRN2", target_bir_lowering=False, debug=False,
                   num_devices=B)
    f32 = mybir.dt.float32
    bf16 = mybir.dt.bfloat16

    x_t = nc.dram_tensor("x", [NPAD, D], bf16, kind="ExternalInput")
    comb_t = nc.dram_tensor("comb", [QP, CW], f32, kind="ExternalInput")
    wz_t = nc.dram_tensor("wz", [NB * WBLK], bf16, kind="ExternalInput")
    out_t = nc.dram_tensor("out", [NOUT, D], bf16, kind="ExternalOutput")

    def shift(t, c0):  # [QP, (i, p)] read of t[q, c0+i+p] (overlapping)
        return bass.AP(t.tensor, t.offset + c0, [t.ap[0], [1, IH], [1, K]])

    def base(t, c0):  # [QP, (i, p-rep)] read of t[q, c0+i]
        return bass.AP(t.tensor, t.offset + c0, [t.ap[0], [1, IH], [0, K]])

    def pb(t):  # [QP, 320] tile -> [QP, i, p] view (p innermost, for reduce)
        return bass.AP(t.tensor, t.offset, [t.ap[0], [K, IH], [1, K]])

    def exp_i(t):  # [QP, IH] tile -> [QP, (i, p-rep)]
        return bass.AP(t.tensor, t.offset, [t.ap[0], [1, IH], [0, K]])

    def bw_rep(t):  # comb bw cols -> [QP, (i-rep, p)]
        return bass.AP(t.tensor, t.offset + 2 * ROWW, [t.ap[0], [0, IH], [1, K]])

    with tile.TileContext(nc) as tc:
        with tc.tile_pool(name="alph", bufs=1) as apool, \
             tc.tile_pool(name="big", bufs=1) as bpool, \
             tc.tile_pool(name="ps", bufs=3, space="PSUM") as pspool:

            # ---- input loads (comb leads the Act queue) ----
            comb = apool.tile([QP, CW], f32)
            nc.scalar.dma_start(comb[:], bass.AP(comb_t, 0, [[CW, QP], [1, CW]]))
            x_all = bpool.tile([128, NB, D], bf16)
            for qe, (j0, nj) in zip((nc.sync, nc.scalar), XCHUNKS):
                qe.dma_start(
                    x_all[:, j0:j0 + nj, :],
                    bass.AP(x_t, j0 * BLK * D,
                            [[D, 128], [BLK * D, nj], [1, D]]))

            # ---- alpha stage on [QP, FA] (Vector + one ACT Exp) ----
            td = apool.tile([QP, FA], f32)
            nc.vector.tensor_tensor(td[:], shift(comb, 0), base(comb, 0),
                                    mybir.AluOpType.subtract)
            cv = apool.tile([QP, FA], f32)
            nc.vector.tensor_tensor(cv[:], shift(comb, ROWW),
                                    base(comb, ROWW), mybir.AluOpType.mult)
            e2 = apool.tile([QP, FA], f32)
            nc.scalar.activation(e2[:], td[:],
                                 mybir.ActivationFunctionType.Exp,
                                 bias=comb[:, CW - 1:CW], scale=-1.0)
            e = apool.tile([QP, FA], f32)
            nc.vector.tensor_tensor(e[:], e2[:], cv[:], mybir.AluOpType.mult)
            cvbw = apool.tile([QP, FA], f32)
            nc.vector.tensor_tensor(cvbw[:], cv[:], bw_rep(comb),
                                    mybir.AluOpType.mult)
            s = apool.tile([QP, IH], f32)
            nc.vector.tensor_reduce(s[:], pb(e), mybir.AxisListType.X,
                                    mybir.AluOpType.add)
            au = apool.tile([QP, FA], f32)
            nc.vector.tensor_tensor(au[:], cvbw[:], exp_i(s[:, :]),
                                    mybir.AluOpType.mult)
            nc.vector.tensor_tensor(au[:], au[:], e[:],
                                    mybir.AluOpType.add)
            sa = apool.tile([QP, IH], f32)
            nc.vector.tensor_reduce(sa[:], pb(au), mybir.AxisListType.X,
                                    mybir.AluOpType.add)
            nc.vector.tensor_scalar(sa[:], sa[:], 1e-8, None,
                                    mybir.AluOpType.max)
            r = apool.tile([QP, IH], f32)
            nc.vector.reciprocal(r[:], sa[:])
            af = apool.tile([QP, FA], bf16)
            nc.vector.tensor_tensor(af[:], au[:], exp_i(r[:, :]),
                                    mybir.AluOpType.mult)

            # ---- pipeline per chunk: skew -> batched xbar -> matmuls ----
            # (skew + xbar share the Sync queue: FIFO gives cheap ordering)
            out_all = bpool.tile([128, NB, D], bf16)
            w_all = bpool.tile([128, NB, KW], bf16)
            evict = [nc.vector.tensor_copy, nc.scalar.copy]
            pts = {}

            def skew(h, j0, nj, qe):
                v = af[h * NB + j0: h * NB + j0 + nj]
                qe.dma_start(
                    bass.AP(wz_t, j0 * WBLK + h * IH * (KW + 1),
                            [[WBLK, nj], [KW + 1, IH], [1, K]]),
                    bass.AP(v.tensor, v.offset,
                            [v.ap[0], [K, IH], [1, K]]))

            def xbar(ci, qe):
                j0, nj = CHUNKS[ci]
                qe.dma_start(
                    w_all[:, j0:j0 + nj, :],
                    bass.AP(wz_t, j0 * WBLK, [[KW, nj * KW], [1, KW]]),
                    transpose=True)

            for ci, (j0, nj) in enumerate(CHUNKS):
                qe = nc.sync if ci % 2 == 0 else nc.scalar
                for h in range(NH):
                    skew(h, j0, nj, qe)
                xbar(ci, nc.sync)
                for jj in range(nj):
                    b = j0 + jj
                    pi = b // 2
                    if b % 2 == 0:
                        pt = pspool.tile([BLK, 2, D], f32, tag="mm",
                                         name=f"pt{pi}")
                        pts[pi] = pt
                    pt = pts[pi]
                    nc.tensor.matmul(pt[:, b % 2, :], w_all[:, b, :BLK],
                                     x_all[:, b, :])
                    if b % 2 == 1:
                        evict[pi % 2](out_all[:BLK, b - 1:b + 1, :], pt[:])
                    elif b == NB - 1:
                        evict[pi % 2](out_all[:BLK, b, :], pt[:, 0, :])
            for si, (j0, nj) in enumerate(SCHUNKS):
                nc.sync.dma_start(
                    bass.AP(out_t, j0 * BLK * D,
                            [[D, BLK], [BLK * D, nj], [1, D]]),
                    out_all[:BLK, j0:j0 + nj, :])
    nc.compile()
    return nc


def _get_nc():
    if "nc" not in _CACHE:
        _CACHE["nc"] = _build()
    return _CACHE["nc"]


def _make_in_maps(x, delta_times, valid_mask, w, beta):
    w64 = w.astype(np.float64)
    wsm = np.exp(w64 - w64.max())
    wsm /= wsm.sum()
    b = 1.0 / (1.0 + np.exp(-float(beta[0])))
    bw = (b / (1.0 - b) * wsm).astype(np.float32)
    wz = np.zeros(NB * WBLK, np.float32).astype(ml_dtypes.bfloat16)

    in_maps = []
    for i in range(B):
        xp = np.zeros((NPAD, D), np.float32)
        xp[:N] = x[i]
        dtp = np.zeros(NPAD, np.float32)
        dtp[:N] = delta_times[i]
        vfp = np.zeros(NPAD, np.float32)
        vfp[:N] = valid_mask[i].astype(np.float32)
        comb = np.zeros((QP, CW), np.float32)
        for h in range(NH):
            for bb in range(NB):
                q = h * NB + bb
                o = bb * BLK + h * IH
                comb[q, 0:ROWW] = dtp[o:o + ROWW]
                comb[q, ROWW:2 * ROWW] = vfp[o:o + ROWW]
                comb[q, 2 * ROWW:2 * ROWW + K] = bw
                comb[q, CW - 1] = EBIAS
        in_maps.append({
            "x": xp.astype(ml_dtypes.bfloat16),
            "comb": comb,
            "wz": wz,
        })
    return in_maps


def _execute(in_maps, trace=False, **kw):
    nc = _get_nc()
    return run_bass_kernel_spmd(nc, in_maps, core_ids=list(range(B)),
                                trace=trace, **kw)


def kernel(x, delta_times, valid_mask, w, beta):
    in_maps = _make_in_maps(x, delta_times, valid_mask, w, beta)
    kr = _execute(in_maps, trace=False)
    outs = [kr.results[i]["out"][:N].astype(np.float32) for i in range(B)]
    return np.stack(outs, axis=0)


# revision 26
# speedup vs baseline: 1.0015x; 1.0015x over previous
"""AdaptiveTokenMixer Trainium2 kernel (8 NeuronCores, pure data parallel).

Per-core algorithm (one batch element per core), pipelined in 4 chunks:
  1. alpha stage runs in [105, 320] layout (partition q = (third h, block b),
     free = (pos i, tap p)); sliding windows are overlapping-stride AP reads
     of a single packed dt/valid/bw row tensor -- no PE transposes.
     exp(-td-12) temporal-decay weights (the bias keeps unmasked pad taps
     finite and cancels in the normalization); the softmax denominator is
     folded into the blend: alpha ~ (e + cv*bw*s) / sum, where
     bw = (b/(1-b))*softmax(w) is host-precomputed.
  2. Three skewed DMAs per chunk (one per third, queues alternate by chunk)
     scatter alpha (bf16) into a DRAM scratch forming the banded
     W^T[m, k] = alpha[n0+m, k-m] per 120-position block.
  3. One DMA-transpose XBAR per chunk (Sync queue only -- the XBAR
     misbehaves on the Act queue) loads its W[k, m] blocks into SBUF;
     one 128x120 @ 128x256 bf16 matmul per block realizes the K-tap mixing
     exactly (PSUM f32, two blocks share a PSUM bank).
  4. Paired PSUM evictions to bf16 staging (DVE/ACT alternating); chunked
     stores overlap the remaining matmuls.
DMA instruction count and order are deliberate: the Tile scheduler
round-robins 8 HWDGE semaphores over all DMAs, so extra or reordered DMAs
couple unrelated phases through semaphore-reuse serialization.

Self-contained: hardcodes shapes for B=8, N=4096, d=256, K=8.
"""
import numpy as np
import ml_dtypes

import concourse.bass as bass
import concourse.bacc as bacc
import concourse.mybir as mybir
from concourse import tile
from concourse.bass_utils import run_bass_kernel_spmd

B, N, D, K = 8, 4096, 256, 8
BLK = 120                      # output positions per block
NB = (N + BLK - 1) // BLK      # 35 blocks -> covers 4200 positions
NOUT = NB * BLK                # 4200 rows in padded device output
NPAD = 4224                    # padded input length
KW = 128                       # k-window (contraction) per block
WBLK = KW * KW                 # W scratch elements per block
NH = 3                         # thirds per block in alpha layout
IH = BLK // NH                 # 40 positions per third
QP = NH * NB                   # 105 partitions used in alpha stage
FA = IH * K                    # 320 free elements per alpha partition
ROWW = IH + K                  # 48: dt/vf row width per third
CW = 2 * ROWW + K + 1          # 105: packed comb row (dt | vf | bw | ebias)
CHUNKS = [(0, 5), (5, 10), (15, 10), (25, 10)]
XCHUNKS = [(0, 18), (18, 17)]            # x load: one chunk per queue
SCHUNKS = [(0, 12), (12, 12), (24, 11)]  # out-store chunking
EBIAS = -12.0                  # exp bias: cancels in softmax, avoids overflow

_CACHE = {}


def _build():
    nc = bacc.Bacc("TRN2", target_bir_lowering=False, debug=False,
                   num_devices=B)
    f32 = mybir.dt.float32
    bf16 = mybir.dt.bfloat16

    x_t = nc.dram_tensor("x", [NPAD, D], bf16, kind="ExternalInput")
    comb_t = nc.dram_tensor("comb", [QP, CW], f32, kind="ExternalInput")
    wz_t = nc.dram_tensor("wz", [NB * WBLK], bf16, kind="ExternalInput")
    out_t = nc.dram_tensor("out", [NOUT, D], bf16, kind="ExternalOutput")

    def shift(t, c0):  # [QP, (i, p)] read of t[q, c0+i+p] (overlapping)
        return bass.AP(t.tensor, t.offset + c0, [t.ap[0], [1, IH], [1, K]])

    def base(t, c0):  # [QP, (i, p-rep)] read of t[q, c0+i]
        return bass.AP(t.tensor, t.offset + c0, [t.ap[0], [1, IH], [0, K]])

    def pb(t):  # [QP, 320] tile -> [QP, i, p] view (p innermost, for reduce)
        return bass.AP(t.tensor, t.offset, [t.ap[0], [K, IH], [1, K]])

    def exp_i(t):  # [QP, IH] tile -> [QP, (i, p-rep)]
        return bass.AP(t.tensor, t.offset, [t.ap[0], [1, IH], [0, K]])

    def bw_rep(t):  # comb bw cols -> [QP, (i-rep, p)]
        return bass.AP(t.tensor, t.offset + 2 * ROWW, [t.ap[0], [0, IH], [1, K]])

    with tile.TileContext(nc) as tc:
        with tc.tile_pool(name="alph", bufs=1) as apool, \
             tc.tile_pool(name="big", bufs=1) as bpool, \
             tc.tile_pool(name="ps", bufs=3, space="PSUM") as pspool:

            # ---- input loads (comb leads the Act queue) ----
            comb = apool.tile([QP, CW], f32)
            nc.scalar.dma_start(comb[:], bass.AP(comb_t, 0, [[CW, QP], [1, CW]]))
            x_all = bpool.tile([128, NB, D], bf16)
            for qe, (j0, nj) in zip((nc.sync, nc.scalar), XCHUNKS):
                qe.dma_start(
                    x_all[:, j0:j0 + nj, :],
                    bass.AP(x_t, j0 * BLK * D,
                            [[D, 128], [BLK * D, nj], [1, D]]))

            # ---- alpha stage on [QP, FA] (Vector + one ACT Exp) ----
            td = apool.tile([QP, FA], f32)
            nc.vector.tensor_tensor(td[:], shift(comb, 0), base(comb, 0),
                                    mybir.AluOpType.subtract)
            cv = apool.tile([QP, FA], f32)
            nc.vector.tensor_tensor(cv[:], shift(comb, ROWW),
                                    base(comb, ROWW), mybir.AluOpType.mult)
            e2 = apool.tile([QP, FA], f32)
            nc.scalar.activation(e2[:], td[:],
                                 mybir.ActivationFunctionType.Exp,
                                 bias=comb[:, CW - 1:CW], scale=-1.0)
            e = apool.tile([QP, FA], f32)
            nc.vector.tensor_tensor(e[:], e2[:], cv[:], mybir.AluOpType.mult)
            cvbw = apool.tile([QP, FA], f32)
            nc.vector.tensor_tensor(cvbw[:], cv[:], bw_rep(comb),
                                    mybir.AluOpType.mult)
            s = apool.tile([QP, IH], f32)
            nc.vector.tensor_reduce(s[:], pb(e), mybir.AxisListType.X,
                                    mybir.AluOpType.add)
            au = apool.tile([QP, FA], f32)
            nc.vector.tensor_tensor(au[:], cvbw[:], exp_i(s[:, :]),
                                    mybir.AluOpType.mult)
            nc.vector.tensor_tensor(au[:], au[:], e[:],
                                    mybir.AluOpType.add)
            sa = apool.tile([QP, IH], f32)
            nc.vector.tensor_reduce(sa[:], pb(au), mybir.AxisListType.X,
                                    mybir.AluOpType.add)
            nc.vector.tensor_scalar(sa[:], sa[:], 1e-8, None,
                                    mybir.AluOpType.max)
            r = apool.tile([QP, IH], f32)
            nc.vector.reciprocal(r[:], sa[:])
            af = apool.tile([QP, FA], bf16)
            nc.vector.tensor_tensor(af[:], au[:], exp_i(r[:, :]),
                                    mybir.AluOpType.mult)

            # ---- pipeline per chunk: skew -> batched xbar -> matmuls ----
            # (skew + xbar share the Sync queue: FIFO gives cheap ordering)
            out_all = bpool.tile([128, NB, D], bf16)
            w_all = bpool.tile([128, NB, KW], bf16)
            evict = [nc.vector.tensor_copy, nc.scalar.copy]
            pts = {}

            def skew(h, j0, nj, qe):
                v = af[h * NB + j0: h * NB + j0 + nj]
                qe.dma_start(
                    bass.AP(wz_t, j0 * WBLK + h * IH * (KW + 1),
                            [[WBLK, nj], [KW + 1, IH], [1, K]]),
                    bass.AP(v.tensor, v.offset,
                            [v.ap[0], [K, IH], [1, K]]))

            def xbar(ci, qe):
                j0, nj = CHUNKS[ci]
                qe.dma_start(
                    w_all[:, j0:j0 + nj, :],
                    bass.AP(wz_t, j0 * WBLK, [[KW, nj * KW], [1, KW]]),
                    transpose=True)

            def run_chunk(ci):
                j0, nj = CHUNKS[ci]
                qe = nc.sync if ci % 2 == 0 else nc.scalar
                for h in range(NH):
                    skew(h, j0, nj, qe)
                xbar(ci, nc.sync)
                for jj in range(nj):
                    b = j0 + jj
                    pi = b // 2
                    if b % 2 == 0:
                        pt = pspool.tile([BLK, 2, D], f32, tag="mm",
                                         name=f"pt{pi}")
                        pts[pi] = pt
                    pt = pts[pi]
                    nc.tensor.matmul(pt[:, b % 2, :], w_all[:, b, :BLK],
                                     x_all[:, b, :])
                    if b % 2 == 1:
                        evict[pi % 2](out_all[:BLK, b - 1:b + 1, :], pt[:])
                    elif b == NB - 1:
                        evict[pi % 2](out_all[:BLK, b, :], pt[:, 0, :])

            for ci in range(len(CHUNKS)):
                run_chunk(ci)
            for si, (j0, nj) in enumerate(SCHUNKS):
                nc.sync.dma_start(
                    bass.AP(out_t, j0 * BLK * D,
                            [[D, BLK], [BLK * D, nj], [1, D]]),
                    out_all[:BLK, j0:j0 + nj, :])
    nc.compile()
    return nc


def _get_nc():
    if "nc" not in _CACHE:
        _CACHE["nc"] = _build()
    return _CACHE["nc"]


def _make_in_maps(x, delta_times, valid_mask, w, beta):
    w64 = w.astype(np.float64)
    wsm = np.exp(w64 - w64.max())
    wsm /= wsm.sum()
    b = 1.0 / (1.0 + np.exp(-float(beta[0])))
    bw = (b / (1.0 - b) * wsm).astype(np.float32)
    wz = np.zeros(NB * WBLK, np.float32).astype(ml_dtypes.bfloat16)

    in_maps = []
    for i in range(B):
        xp = np.zeros((NPAD, D), np.float32)
        xp[:N] = x[i]
        dtp = np.zeros(NPAD, np.float32)
        dtp[:N] = delta_times[i]
        vfp = np.zeros(NPAD, np.float32)
        vfp[:N] = valid_mask[i].astype(np.float32)
        comb = np.zeros((QP, CW), np.float32)
        for h in range(NH):
            for bb in range(NB):
                q = h * NB + bb
                o = bb * BLK + h * IH
                comb[q, 0:ROWW] = dtp[o:o + ROWW]
                comb[q, ROWW:2 * ROWW] = vfp[o:o + ROWW]
                comb[q, 2 * ROWW:2 * ROWW + K] = bw
                comb[q, CW - 1] = EBIAS
        in_maps.append({
            "x": xp.astype(ml_dtypes.bfloat16),
            "comb": comb,
            "wz": wz,
        })
    return in_maps


def _execute(in_maps, trace=False, **kw):
    nc = _get_nc()
    return run_bass_kernel_spmd(nc, in_maps, core_ids=list(range(B)),
                                trace=trace, **kw)


def kernel(x, delta_times, valid_mask, w, beta):
    in_maps = _make_in_maps(x, delta_times, valid_mask, w, beta)
    kr = _execute(in_maps, trace=False)
    outs = [kr.results[i]["out"][:N].astype(np.float32) for i in range(B)]
    return np.stack(outs, axis=0)
